# revision 1
# baseline (speedup 1.0000x reference)
"""Trainium2 distributed causal attention kernel (8 NeuronCores).

Problem: x[4,2048,1024] -> qkv proj -> 16-head causal attention -> out proj.

Sharding (uniform SPMD graph on all 8 cores):
  core c = (batch b = c//2, head-group g = c%2 of 8 heads).
  Each core: projects q/k/v for its 8 heads over the full 2048 tokens of its
  batch, runs causal flash-style attention (no max subtraction -- scores are
  O(1) for this input distribution), computes the partial output projection
  with its 512 inner dims of w_out, adds b_out/2, then a pairwise
  ReduceScatter(add) over {2b, 2b+1} yields final output token-stripes.
  Host reassembles stripes. No other collectives.

Layouts (all matmuls are layout-natural; x is transposed on the host):
  xT   [1024(dm), 2048(tok)]  f32r   (moving operand of kT/qT proj, stationary of v proj)
  kT,qT [512(inner) as 4x[128], 2048] bf16 (keys/queries transposed, 2 heads per tile)
  v_aug [2048(tok) as 16x[128], 8*65] bf16 (per head: 64 v-cols + ones col -> softmax denom)
  simT psum [128(key), 512(tok)] = k-block^T @ q-chunk   (K=64, heads packed 2x in PE array)
  pT = exp(simT * 0.125) bf16, causal band masks applied multiplicatively
  pv psum [65, 512] accumulates over k-blocks (row 64 = denominator)
  attnoutT bf16 [512(inner), 512(tok)] per chunk -> out-proj psum [128(tok), 512(col)]
"""

import sys

sys.path.insert(0, "/opt/trn_rl_repo")

import numpy as np

B, N, DM = 4, 2048, 1024
H, DH = 16, 64
HG = 8  # heads per core
LI = HG * DH  # local inner = 512
NCORES = 8
CHUNK = 512  # q-chunk tokens
NCHUNK = N // CHUNK  # 4
KB = 128  # k-block size
VW = DH + 1  # v columns per head incl. ones column

_GRAPH = None


def _build_graph(dbg=False):
    from concourse import bacc, bass, mybir, tile

    f32 = mybir.dt.float32
    f32r = mybir.dt.float32r
    bf16 = mybir.dt.bfloat16
    Exp = mybir.ActivationFunctionType.Exp

    nc = bacc.Bacc("TRN2", target_bir_lowering=False, debug=False)

    xT_d = nc.dram_tensor("xT", [DM, N], f32r, kind="ExternalInput")
    wq_d = nc.dram_tensor("wq", [DM, LI], f32r, kind="ExternalInput")
    wk_d = nc.dram_tensor("wk", [DM, LI], f32r, kind="ExternalInput")
    wv_d = nc.dram_tensor("wv", [DM, LI], f32r, kind="ExternalInput")
    wo_d = nc.dram_tensor("wo", [LI, DM], f32, kind="ExternalInput")
    hb_d = nc.dram_tensor("hb", [1, DM], f32, kind="ExternalInput")
    mask_d = nc.dram_tensor("mask", [KB, KB], bf16, kind="ExternalInput")
    out_d = nc.dram_tensor("out", [N // 2, DM], f32, kind="ExternalOutput")
    if dbg:
        dkT = nc.dram_tensor("dkT", [128, N], f32, kind="ExternalOutput")
        dqT = nc.dram_tensor("dqT", [128, N], f32, kind="ExternalOutput")
        dva = nc.dram_tensor("dva", [128, HG * VW], f32, kind="ExternalOutput")
        dpt = nc.dram_tensor("dpt", [128, CHUNK], f32, kind="ExternalOutput")
        dpv = nc.dram_tensor("dpv", [VW, CHUNK], f32, kind="ExternalOutput")
        dao = nc.dram_tensor("dao", [128, CHUNK], f32, kind="ExternalOutput")
        dpd = nc.dram_tensor("dpd", [CHUNK, DM], f32, kind="ExternalOutput")

    RG = [[0, 1], [2, 3], [4, 5], [6, 7]]

    with tile.TileContext(nc) as tc:
        with (
            tc.tile_pool(name="persist", bufs=1) as pers,
            tc.tile_pool(name="projtmp", bufs=1) as ptmp,
            tc.tile_pool(name="wstream", bufs=3) as wstr,
            tc.tile_pool(name="work", bufs=4) as work,
            tc.tile_pool(name="aux", bufs=2) as aux,
            tc.tile_pool(name="mmps", bufs=2, space="PSUM") as mmps,
            tc.tile_pool(name="simps", bufs=4, space="PSUM") as simps,
            tc.tile_pool(name="pvps", bufs=2, space="PSUM") as pvps,
            tc.tile_pool(name="dram", bufs=2, space="DRAM") as dram,
        ):
            # ---- constants / persistent tiles ----
            mask_sb = pers.tile([KB, KB], bf16, tag="mask")
            nc.sync.dma_start(out=mask_sb[:, :], in_=mask_d[:, :])

            ones_bf = pers.tile([1, KB], bf16, tag="ones")
            nc.vector.memset(ones_bf[:, :], 1.0)

            hb_f = aux.tile([1, DM], f32, tag="hbf")
            nc.sync.dma_start(out=hb_f[:, :], in_=hb_d[:, :])
            hb_bf = pers.tile([1, DM], bf16, tag="hbb")
            nc.vector.tensor_copy(hb_bf[:, :], hb_f[:, :])

            wo_bf = []
            for it in range(4):
                wof = aux.tile([128, DM], f32, tag="wof", bufs=1)
                nc.sync.dma_start(out=wof[:, :], in_=wo_d[it * 128 : (it + 1) * 128, :])
                wob = pers.tile([128, DM], bf16, tag=f"wo{it}")
                nc.vector.tensor_copy(wob[:, :], wof[:, :])
                wo_bf.append(wob)

            # ---- phase 1: projections ----
            xT = []
            for d in range(8):
                t = ptmp.tile([128, N], f32r, tag=f"xT{d}")
                for cc in range(4):
                    nc.sync.dma_start(
                        out=t[:, cc * 512 : (cc + 1) * 512],
                        in_=xT_d[d * 128 : (d + 1) * 128, cc * 512 : (cc + 1) * 512],
                    )
                xT.append(t)

            kT = [pers.tile([128, N], bf16, tag=f"kT{i}", name=f"kT{i}") for i in range(4)]
            qT = [pers.tile([128, N], bf16, tag=f"qT{i}", name=f"qT{i}") for i in range(4)]

            for w_d, dst in ((wk_d, kT), (wq_d, qT)):
                for it in range(4):
                    wt = []
                    for d in range(8):
                        t = wstr.tile([128, 128], f32r, tag="wt", bufs=8)
                        nc.sync.dma_start(
                            out=t[:, :],
                            in_=w_d[d * 128 : (d + 1) * 128, it * 128 : (it + 1) * 128],
                        )
                        wt.append(t)
                    for tt in range(4):
                        ps = mmps.tile([128, 512], f32, tag="mm")
                        for d in range(8):
                            nc.tensor.matmul(
                                ps[:, :],
                                lhsT=wt[d][:, :],
                                rhs=xT[d][:, tt * 512 : (tt + 1) * 512],
                                start=(d == 0),
                                stop=(d == 7),
                            )
                        nc.vector.tensor_copy(
                            dst[it][:, tt * 512 : (tt + 1) * 512], ps[:, :]
                        )

            wv = []
            for d in range(8):
                t = ptmp.tile([128, LI], f32r, tag=f"wv{d}")
                nc.sync.dma_start(out=t[:, :], in_=wv_d[d * 128 : (d + 1) * 128, :])
                wv.append(t)

            v_aug = [pers.tile([128, HG * VW], bf16, tag=f"va{t}", name=f"va{t}") for t in range(16)]
            for tt in range(16):
                va3 = v_aug[tt].rearrange("p (h c) -> p h c", h=HG)
                nc.vector.memset(va3[:, :, DH : DH + 1], 1.0)
                ps = mmps.tile([128, 512], f32, tag="mm")
                for d in range(8):
                    nc.tensor.matmul(
                        ps[:, :],
                        lhsT=xT[d][:, tt * 128 : (tt + 1) * 128],
                        rhs=wv[d][:, :],
                        start=(d == 0),
                        stop=(d == 7),
                    )
                nc.vector.tensor_copy(
                    va3[:, :, 0:DH], ps.rearrange("p (h c) -> p h c", h=HG)
                )

            if dbg:
                for src_t, dst_d in ((kT[0], dkT), (qT[0], dqT)):
                    for pc in range(4):
                        dc = aux.tile([128, 512], f32, tag="dbgc", bufs=1)
                        nc.vector.tensor_copy(dc[:, :], src_t[:, pc * 512 : (pc + 1) * 512])
                        nc.sync.dma_start(out=dst_d[:, pc * 512 : (pc + 1) * 512], in_=dc[:, :])
                dc = aux.tile([128, HG * VW], f32, tag="dbgc2", bufs=1)
                nc.vector.tensor_copy(dc[:, :], v_aug[0][:, :])
                nc.sync.dma_start(out=dva[:, :], in_=dc[:, :])

            # ---- phases 2+3: attention + out-proj + RS, chunk-pipelined ----
            # outproj of chunk c is emitted AFTER attention of chunk c+1 so the
            # PE FIFO never stalls on c's normalize epilogue (DVE/DMA chain).
            chunk_aos = {}

            def attention_chunk(c):
                nk = 4 * (c + 1)
                aos = [
                    work.tile(
                        [128, CHUNK], bf16, tag=f"ao{i}", name=f"ao{i}", bufs=2
                    )
                    for i in range(4)
                ]
                dn = work.tile([8, CHUNK], f32, tag="dn", name="dn", bufs=2)
                vals = [None] * 8
                for hp in range(4):
                    pvs = [
                        pvps.tile([VW, CHUNK], f32, tag="pv", name="pv")
                        for _ in range(2)
                    ]
                    sims_of = {}

                    def qk_step(jb):
                        sims = [
                            simps.tile([128, CHUNK], f32, tag="sim", name="sim")
                            for _ in range(2)
                        ]
                        for e in range(2):
                            nc.tensor.matmul(
                                sims[e][:, :],
                                lhsT=kT[hp][
                                    64 * e : 64 * e + 64, jb * KB : (jb + 1) * KB
                                ],
                                rhs=qT[hp][
                                    64 * e : 64 * e + 64, c * CHUNK : (c + 1) * CHUNK
                                ],
                                start=True,
                                stop=True,
                            )
                        sims_of[jb] = sims

                    def pv_step(jb):
                        sims = sims_of.pop(jb)
                        v = jb - (nk - 4)
                        col0 = max(0, v) * KB
                        for e in range(2):
                            h = 2 * hp + e
                            pt = work.tile([128, CHUNK], bf16, tag=f"pt{e}", bufs=3)
                            if col0 > 0:
                                nc.vector.memset(pt[:, 0:col0], 0.0)
                            nc.scalar.activation(
                                pt[:, col0:CHUNK],
                                sims[e][:, col0:CHUNK],
                                Exp,
                                scale=float(DH**-0.5),
                            )
                            if v >= 0:
                                nc.vector.tensor_mul(
                                    pt[:, col0 : col0 + KB],
                                    pt[:, col0 : col0 + KB],
                                    mask_sb[:, :],
                                )
                            nc.tensor.matmul(
                                pvs[e][:, :],
                                lhsT=v_aug[jb][:, h * VW : (h + 1) * VW],
                                rhs=pt[:, :],
                                start=(jb == 0),
                                stop=(jb == nk - 1),
                            )

                    qk_step(0)
                    for jb in range(1, nk):
                        qk_step(jb)
                        pv_step(jb - 1)
                    pv_step(nk - 1)

                    # evacuate PV psum to SBUF immediately so the psum banks
                    # free for the next head-pair; denominators collect into dn
                    for e in range(2):
                        h = 2 * hp + e
                        t = work.tile(
                            [DH, CHUNK], bf16, tag=f"pvsb{h}", bufs=2, name=f"pvsb{h}"
                        )
                        nc.vector.tensor_copy(t[:, :], pvs[e][0:DH, :])
                        vals[h] = t
                        tmpd = work.tile([1, CHUNK], f32, tag="tmpd", bufs=2)
                        nc.vector.tensor_copy(tmpd[:, :], pvs[e][DH : DH + 1, :])
                        nc.sync.dma_start(out=dn[h : h + 1, :], in_=tmpd[:, :])

                chunk_aos[c] = (aos, vals, dn)

            def epilogue_chunk(c):
                aos, vals, dn = chunk_aos[c]
                rc = work.tile([8, CHUNK], f32, tag="rc", name="rc", bufs=2)
                nc.vector.reciprocal(rc[:, :], dn[:, :])
                rcb = work.tile([8, CHUNK], bf16, tag="rcb", name="rcb", bufs=2)
                nc.vector.tensor_copy(rcb[:, :], rc[:, :])
                for h in range(8):
                    rb = work.tile([64, CHUNK], bf16, tag="rb", bufs=4)
                    rrow = rcb[h : h + 1, :]
                    rsrc = bass.AP(
                        tensor=rrow.tensor,
                        offset=rrow.offset,
                        ap=[[CHUNK, 1], [0, 64], [1, CHUNK]],
                    )
                    nc.sync.dma_start(out=rb[:, :], in_=rsrc)
                    nc.vector.tensor_mul(
                        aos[h // 2][64 * (h % 2) : 64 * (h % 2) + 64, :],
                        vals[h][:, :],
                        rb[:, :],
                    )

            def outproj_chunk(c, n_rs=1):
                aos, _, _ = chunk_aos.pop(c)
                pd = dram.tile([CHUNK, DM], f32, tag="pd")
                ts_per_rs = 4 // n_rs
                for rs_i in range(n_rs):
                    for ts in range(rs_i * ts_per_rs, (rs_i + 1) * ts_per_rs):
                        for ct in range(2):
                            po = mmps.tile([128, 512], f32, tag="mm")
                            nc.tensor.matmul(
                                po[:, :],
                                lhsT=ones_bf[:, :],
                                rhs=hb_bf[:, ct * 512 : (ct + 1) * 512],
                                start=True,
                                stop=False,
                            )
                            for it in range(4):
                                nc.tensor.matmul(
                                    po[:, :],
                                    lhsT=aos[it][:, ts * 128 : (ts + 1) * 128],
                                    rhs=wo_bf[it][:, ct * 512 : (ct + 1) * 512],
                                    start=False,
                                    stop=(it == 3),
                                )
                            ob = work.tile([128, 512], f32, tag="ob", name="ob", bufs=2)
                            nc.vector.tensor_copy(ob[:, :], po[:, :])
                            nc.sync.dma_start(
                                out=pd[
                                    ts * 128 : (ts + 1) * 128,
                                    ct * 512 : (ct + 1) * 512,
                                ],
                                in_=ob[:, :],
                            )
                    rows = CHUNK // n_rs
                    rs = dram.tile(
                        [rows // 2, DM], f32, tag="rs", name="rs", padded_shape=[CHUNK // 2, DM]
                    )
                    nc.gpsimd.collective_compute(
                        "ReduceScatter",
                        mybir.AluOpType.add,
                        replica_groups=RG,
                        ins=[pd[rs_i * rows : (rs_i + 1) * rows, :].opt()],
                        outs=[rs[:, :].opt()],
                    )
                    out_r0 = c * 256 + rs_i * (rows // 2)
                    nc.sync.dma_start(
                        out=out_d[out_r0 : out_r0 + rows // 2, :], in_=rs[:, :]
                    )

            attention_chunk(0)
            for c in range(1, NCHUNK):
                epilogue_chunk(c - 1)
                attention_chunk(c)
                outproj_chunk(c - 1)
            epilogue_chunk(NCHUNK - 1)
            outproj_chunk(NCHUNK - 1, n_rs=2)

    nc.finalize()
    return nc


def _get_graph():
    global _GRAPH
    if _GRAPH is None:
        _GRAPH = _build_graph()
    return _GRAPH


def _build_masks():
    # [j, ti] = 1 where ti >= j: token ti attends key j within the diagonal block
    return np.ascontiguousarray(np.triu(np.ones((KB, KB), np.float32)))


def _make_in_maps(x, w_qkv, w_out, b_out):
    x = np.asarray(x, np.float32)
    w_qkv = np.asarray(w_qkv, np.float32)
    w_out = np.asarray(w_out, np.float32)
    b_out = np.asarray(b_out, np.float32)
    import ml_dtypes

    xT = [np.ascontiguousarray(x[b].T) for b in range(B)]
    masks = _build_masks().astype(ml_dtypes.bfloat16)
    hb = np.ascontiguousarray((0.5 * b_out).reshape(1, DM))
    in_maps = []
    for c in range(NCORES):
        b, g = c // 2, c % 2
        in_maps.append(
            {
                "xT": xT[b],
                "wq": np.ascontiguousarray(w_qkv[:, LI * g : LI * (g + 1)]),
                "wk": np.ascontiguousarray(w_qkv[:, DM + LI * g : DM + LI * (g + 1)]),
                "wv": np.ascontiguousarray(
                    w_qkv[:, 2 * DM + LI * g : 2 * DM + LI * (g + 1)]
                ),
                "wo": np.ascontiguousarray(w_out[LI * g : LI * (g + 1), :]),
                "hb": hb,
                "mask": masks,
            }
        )
    return in_maps


def _assemble(results):
    y = np.empty((B, N, DM), np.float32)
    for c in range(NCORES):
        b, g = c // 2, c % 2
        o = results[c]["out"]  # [1024, 1024] of token stripes
        for ch in range(NCHUNK - 1):
            t0 = ch * CHUNK + g * 256
            y[b, t0 : t0 + 256] = o[ch * 256 : (ch + 1) * 256]
        ch = NCHUNK - 1  # last chunk: two half-size ReduceScatter pieces
        for p in range(2):
            t0 = ch * CHUNK + p * 256 + g * 128
            r0 = ch * 256 + p * 128
            y[b, t0 : t0 + 128] = o[r0 : r0 + 128]
    return y


def _install_ntff_hook_shim():
    """The container's antenv package lacks axon_hooks; synthesize it so
    run_bass_kernel_spmd(trace=True) can NTFF-profile via the injected .so."""
    import types

    if "antenv.axon_hooks" in sys.modules:
        return
    try:
        from trn_agent_boot.trn_boot import _ntff_profile_via_ctypes

        hook = _ntff_profile_via_ctypes("/opt/axon/libaxon_pjrt.so")
    except Exception as e:  # profiling degrades, run still works
        print(f"ntff hook shim unavailable: {e}")
        hook = None
    mod = types.ModuleType("antenv.axon_hooks")
    _state = {"hook": hook}
    mod.set_axon_ntff_profile_hook = lambda h: _state.__setitem__("hook", h)
    mod.get_axon_ntff_profile_hook = lambda: _state["hook"]
    sys.modules["antenv.axon_hooks"] = mod
    import antenv

    antenv.axon_hooks = mod


def _run(in_maps, trace=False):
    from concourse import bass_utils

    if trace:
        _install_ntff_hook_shim()
    nc = _get_graph()
    return bass_utils.run_bass_kernel_spmd(
        nc, in_maps, core_ids=list(range(NCORES)), trace=trace
    )


def kernel(x, w_qkv, w_out, b_out):
    res = _run(_make_in_maps(x, w_qkv, w_out, b_out), trace=False)
    return _assemble(res.results)


def kernel_timed(x, w_qkv, w_out, b_out):
    res = _run(_make_in_maps(x, w_qkv, w_out, b_out), trace=True)
    return _assemble(res.results), res



# revision 4
# speedup vs baseline: 1.3590x; 1.3590x over previous
"""Trainium2 distributed causal attention kernel (8 NeuronCores).

Problem: x[4,2048,1024] -> qkv proj -> 16-head causal attention -> out proj.

Sharding (uniform SPMD graph on all 8 cores):
  core c = (batch b = c//2, head-group g = c%2 of 8 heads).
  Each core: projects q/k/v for its 8 heads over the full 2048 tokens of its
  batch (all matmul inputs bf16, host-cast), runs causal flash-style attention
  (no max subtraction -- scores are O(1) for this input distribution),
  computes the partial output projection with its 512 inner dims of w_out,
  adds b_out/2, then a pairwise ReduceScatter(add) over {2b, 2b+1} yields
  final output token-stripes. Host reassembles stripes.

Pipeline: the projection of chunk c+1 and out-projection of chunk c-1 are
interleaved as PE "filler" work inside attention of chunk c, so the PE never
waits on the Scalar engine's EXP chain and stays at full p-state.

Layouts:
  xT   [1024(dm), 2048(tok)] bf16, loaded per 512-token chunk
  kT,qT [512(inner) as 4x[128], 2048] bf16 (2 heads per 128-partition tile)
  v_aug [2048(tok) as 16x[128], 8*65] bf16 (per head: 64 v-cols + ones col)
  sims psum [128(key), 1024(= 2 heads x 512 tok)] = one EXP per head-pair
  pt = exp(sims * 0.125) bf16 [128, 1024], causal masks multiplicative
  pv psum [65, 512] per head accumulates over k-blocks (row 64 = denominator)
  out-proj psum [128(tok), 512(dm col)], bias added during DVE evacuation
"""

import sys

sys.path.insert(0, "/opt/trn_rl_repo")

import numpy as np

B, N, DM = 4, 2048, 1024
H, DH = 16, 64
HG = 8  # heads per core
LI = HG * DH  # local inner = 512
NCORES = 8
CHUNK = 512  # q-chunk tokens
NCHUNK = N // CHUNK  # 4
KB = 128  # k-block size
VW = DH + 1  # v columns per head incl. ones column

_GRAPH = None


def _build_graph():
    from concourse import bacc, bass, mybir, tile

    f32 = mybir.dt.float32
    bf16 = mybir.dt.bfloat16
    Exp = mybir.ActivationFunctionType.Exp

    nc = bacc.Bacc("TRN2", target_bir_lowering=False, debug=False)

    xT_d = nc.dram_tensor("xT", [DM, N], bf16, kind="ExternalInput")
    wq_d = nc.dram_tensor("wq", [DM, LI], bf16, kind="ExternalInput")
    wk_d = nc.dram_tensor("wk", [DM, LI], bf16, kind="ExternalInput")
    wv_d = nc.dram_tensor("wv", [DM, LI], bf16, kind="ExternalInput")
    wo_d = nc.dram_tensor("wo", [LI, DM], bf16, kind="ExternalInput")
    hb_d = nc.dram_tensor("hb", [1, DM], f32, kind="ExternalInput")
    mask_d = nc.dram_tensor("mask", [KB, KB], bf16, kind="ExternalInput")
    out_d = nc.dram_tensor("out", [N // 2, DM], f32, kind="ExternalOutput")

    RG = [[0, 1], [2, 3], [4, 5], [6, 7]]

    with tile.TileContext(nc) as tc:
        with (
            tc.tile_pool(name="persist", bufs=1) as pers,
            tc.tile_pool(name="work", bufs=4) as work,
            tc.tile_pool(name="aux", bufs=2) as aux,
            tc.tile_pool(name="mmps", bufs=2, space="PSUM") as mmps,
            tc.tile_pool(name="simps", bufs=2, space="PSUM") as simps,
            tc.tile_pool(name="pvps", bufs=2, space="PSUM") as pvps,
            tc.tile_pool(name="dram", bufs=2, space="DRAM") as dram,
        ):
            # ---- persistent constants / weights (DMAs issued up front) ----
            wk_sb = [pers.tile([128, LI], bf16, tag=f"wk{d}", name=f"wk{d}") for d in range(8)]
            wq_sb = [pers.tile([128, LI], bf16, tag=f"wq{d}", name=f"wq{d}") for d in range(8)]
            wv_sb = [pers.tile([128, LI], bf16, tag=f"wv{d}", name=f"wv{d}") for d in range(8)]
            xc = [
                [pers.tile([128, CHUNK], bf16, tag=f"x{c}_{d}", name=f"x{c}_{d}") for d in range(8)]
                for c in range(NCHUNK)
            ]

            # first the tiles needed by proj(0), in consumption order
            for d in range(8):
                nc.sync.dma_start(
                    out=wk_sb[d][:, :], in_=wk_d[d * 128 : (d + 1) * 128, :]
                )
            for d in range(8):
                nc.sync.dma_start(
                    out=xc[0][d][:, :], in_=xT_d[d * 128 : (d + 1) * 128, 0:CHUNK]
                )
            for d in range(8):
                nc.sync.dma_start(
                    out=wq_sb[d][:, :], in_=wq_d[d * 128 : (d + 1) * 128, :]
                )
            for d in range(8):
                nc.sync.dma_start(
                    out=wv_sb[d][:, :], in_=wv_d[d * 128 : (d + 1) * 128, :]
                )

            mask_sb = pers.tile([KB, KB], bf16, tag="mask", name="mask")
            nc.sync.dma_start(out=mask_sb[:, :], in_=mask_d[:, :])

            wo_bf = []
            for it in range(4):
                wob = pers.tile([128, DM], bf16, tag=f"wo{it}", name=f"wo{it}")
                nc.sync.dma_start(out=wob[:, :], in_=wo_d[it * 128 : (it + 1) * 128, :])
                wo_bf.append(wob)

            hb_f = aux.tile([1, DM], f32, tag="hbf", bufs=1, name="hbf")
            nc.sync.dma_start(out=hb_f[:, :], in_=hb_d[:, :])
            # broadcast bias to all 128 partitions once (DMA replication trick)
            bias_bc = pers.tile([128, DM], f32, tag="biasbc", name="biasbc")
            hrow = hb_f[0:1, :]
            hsrc = bass.AP(
                tensor=hrow.tensor,
                offset=hrow.offset,
                ap=[[DM, 1], [0, 128], [1, DM]],
            )
            nc.sync.dma_start(out=bias_bc[:, :], in_=hsrc)

            kT = [pers.tile([128, N], bf16, tag=f"kT{i}", name=f"kT{i}") for i in range(4)]
            qT = [pers.tile([128, N], bf16, tag=f"qT{i}", name=f"qT{i}") for i in range(4)]
            v_aug = [
                pers.tile([128, HG * VW], bf16, tag=f"va{t}", name=f"va{t}")
                for t in range(16)
            ]

            # ---- emission helpers ----
            def xc_dma(c):
                for d in range(8):
                    nc.gpsimd.dma_start(
                        out=xc[c][d][:, :],
                        in_=xT_d[d * 128 : (d + 1) * 128, c * CHUNK : (c + 1) * CHUNK],
                    )

            def proj_fillers(c):
                """k/q/v projection of chunk c as a list of PE filler closures
                (each ~4 matmuls; evac on the closing half)."""
                out = []
                for w_sb, dst in ((wk_sb, kT), (wq_sb, qT)):
                    for it in range(4):

                        def f0(w_sb=w_sb, it=it):
                            ps = mmps.tile([128, CHUNK], f32, tag="mm", name="mm")
                            for d in range(4):
                                nc.tensor.matmul(
                                    ps[:, :],
                                    lhsT=w_sb[d][:, it * 128 : (it + 1) * 128],
                                    rhs=xc[c][d][:, :],
                                    start=(d == 0),
                                    stop=False,
                                )
                            return ps

                        def f1(ps_ref, w_sb=w_sb, it=it, dst=dst):
                            ps = ps_ref[0]
                            for d in range(4, 8):
                                nc.tensor.matmul(
                                    ps[:, :],
                                    lhsT=w_sb[d][:, it * 128 : (it + 1) * 128],
                                    rhs=xc[c][d][:, :],
                                    start=False,
                                    stop=(d == 7),
                                )
                            nc.vector.tensor_copy(
                                dst[it][:, c * CHUNK : (c + 1) * CHUNK], ps[:, :]
                            )

                        out.append((f0, f1))
                for tt4 in range(4):
                    tt = 4 * c + tt4

                    def g0(tt4=tt4):
                        ps = mmps.tile([128, CHUNK], f32, tag="mm", name="mm")
                        for d in range(4):
                            nc.tensor.matmul(
                                ps[:, :],
                                lhsT=xc[c][d][:, tt4 * 128 : (tt4 + 1) * 128],
                                rhs=wv_sb[d][:, :],
                                start=(d == 0),
                                stop=False,
                            )
                        return ps

                    def g1(ps_ref, tt=tt, tt4=tt4):
                        ps = ps_ref[0]
                        for d in range(4, 8):
                            nc.tensor.matmul(
                                ps[:, :],
                                lhsT=xc[c][d][:, tt4 * 128 : (tt4 + 1) * 128],
                                rhs=wv_sb[d][:, :],
                                start=False,
                                stop=(d == 7),
                            )
                        va3 = v_aug[tt].rearrange("p (h c) -> p h c", h=HG)
                        nc.vector.memset(va3[:, :, DH : DH + 1], 1.0)
                        nc.vector.tensor_copy(
                            va3[:, :, 0:DH], ps.rearrange("p (h c) -> p h c", h=HG)
                        )

                    out.append((g0, g1))
                # flatten into single-closure units with shared psum handoff
                units = []
                for f0, f1 in out:
                    ps_ref = [None]

                    def u0(f0=f0, ps_ref=ps_ref):
                        ps_ref[0] = f0()

                    def u1(f1=f1, ps_ref=ps_ref):
                        f1(ps_ref)

                    units.append(u0)
                    units.append(u1)
                return units

            chunk_state = {}

            def outproj_fillers(c, n_rs=1):
                """out-projection + RS of chunk c as filler closures."""
                aos = chunk_state[c]
                pd = dram.tile([CHUNK, DM], f32, tag="pd", name="pd")
                units = []
                ts_per_rs = 4 // n_rs
                for rs_i in range(n_rs):
                    for ts in range(rs_i * ts_per_rs, (rs_i + 1) * ts_per_rs):
                        for ct in range(2):

                            def u(ts=ts, ct=ct):
                                po = mmps.tile([128, CHUNK], f32, tag="mm", name="mm")
                                for it in range(4):
                                    nc.tensor.matmul(
                                        po[:, :],
                                        lhsT=aos[it][:, ts * 128 : (ts + 1) * 128],
                                        rhs=wo_bf[it][:, ct * 512 : (ct + 1) * 512],
                                        start=(it == 0),
                                        stop=(it == 3),
                                    )
                                ob = work.tile(
                                    [128, 512], f32, tag="ob", name="ob", bufs=2
                                )
                                nc.vector.tensor_add(
                                    ob[:, :], po[:, :],
                                    bias_bc[:, ct * 512 : (ct + 1) * 512],
                                )
                                nc.gpsimd.dma_start(
                                    out=pd[
                                        ts * 128 : (ts + 1) * 128,
                                        ct * 512 : (ct + 1) * 512,
                                    ],
                                    in_=ob[:, :],
                                )

                            units.append(u)

                    def urs(rs_i=rs_i):
                        rows = CHUNK // n_rs
                        rs = dram.tile(
                            [rows // 2, DM], f32, tag="rs", name="rs",
                            padded_shape=[CHUNK // 2, DM],
                        )
                        nc.gpsimd.collective_compute(
                            "ReduceScatter",
                            mybir.AluOpType.add,
                            replica_groups=RG,
                            ins=[pd[rs_i * rows : (rs_i + 1) * rows, :].opt()],
                            outs=[rs[:, :].opt()],
                        )
                        out_r0 = c * 256 + rs_i * (rows // 2)
                        nc.sync.dma_start(
                            out=out_d[out_r0 : out_r0 + rows // 2, :], in_=rs[:, :]
                        )

                    units.append(urs)
                return units

            def attention_chunk(c, fillers):
                """Causal attention for chunk c; pops filler closures between
                QK and PV emissions to keep the PE dense."""
                nk = 4 * (c + 1)
                nsteps = 4 * nk
                step = [0]

                def drain():
                    step[0] += 1
                    left = nsteps - step[0]
                    k = -(-len(fillers) // (left + 1))  # ceil: spread evenly
                    for _ in range(min(k, len(fillers))):
                        fillers.pop(0)()

                aos = [
                    work.tile([128, CHUNK], bf16, tag=f"ao{i}", name=f"ao{i}", bufs=2)
                    for i in range(4)
                ]
                vals = [None] * 8
                for hp in range(4):
                    pvs = [
                        pvps.tile([VW, CHUNK], f32, tag="pv", name="pv")
                        for _ in range(2)
                    ]
                    sims_of = {}

                    def qk_step(jb):
                        v = jb - (nk - 4)
                        col0 = max(0, v) * KB
                        sims = simps.tile([128, 2 * CHUNK], f32, tag="sim", name="sim")
                        for e in range(2):
                            nc.tensor.matmul(
                                sims[:, e * CHUNK + col0 : (e + 1) * CHUNK],
                                lhsT=kT[hp][
                                    64 * e : 64 * e + 64, jb * KB : (jb + 1) * KB
                                ],
                                rhs=qT[hp][
                                    64 * e : 64 * e + 64,
                                    c * CHUNK + col0 : (c + 1) * CHUNK,
                                ],
                                start=True,
                                stop=True,
                            )
                        sims_of[jb] = (sims, col0)

                    def exp_step(jb):
                        sims, col0 = sims_of[jb]
                        pt = work.tile(
                            [128, 2 * CHUNK], bf16, tag="pt", name="pt", bufs=3
                        )
                        if col0 > 0:
                            pt3 = pt.rearrange("p (e t) -> p e t", e=2)
                            nc.vector.memset(pt3[:, :, 0:col0], 0.0)
                            s3 = sims.rearrange("p (e t) -> p e t", e=2)
                            nc.scalar.activation(
                                pt3[:, :, col0:CHUNK],
                                s3[:, :, col0:CHUNK],
                                Exp,
                                scale=float(DH**-0.5),
                            )
                        else:
                            nc.scalar.activation(
                                pt[:, :], sims[:, :], Exp, scale=float(DH**-0.5)
                            )
                        v = jb - (nk - 4)
                        if v >= 0:
                            for e in range(2):
                                nc.vector.tensor_mul(
                                    pt[:, e * CHUNK + col0 : e * CHUNK + col0 + KB],
                                    pt[:, e * CHUNK + col0 : e * CHUNK + col0 + KB],
                                    mask_sb[:, :],
                                )
                        sims_of[jb] = pt

                    def pv_step(jb):
                        pt = sims_of.pop(jb)
                        for e in range(2):
                            h = 2 * hp + e
                            nc.tensor.matmul(
                                pvs[e][:, :],
                                lhsT=v_aug[jb][:, h * VW : (h + 1) * VW],
                                rhs=pt[:, e * CHUNK : (e + 1) * CHUNK],
                                start=(jb == 0),
                                stop=(jb == nk - 1),
                            )

                    qk_step(0)
                    exp_step(0)
                    for jb in range(1, nk):
                        qk_step(jb)
                        exp_step(jb)
                        drain()
                        pv_step(jb - 1)
                    drain()
                    pv_step(nk - 1)

                    # evacuate PV psum (values + denominator row) to SBUF
                    for e in range(2):
                        h = 2 * hp + e
                        t = work.tile(
                            [VW, CHUNK], bf16, tag=f"pvsb{h}", bufs=2, name=f"pvsb{h}"
                        )
                        nc.vector.tensor_copy(t[:, :], pvs[e][:, :])
                        vals[h] = t

                chunk_state[c] = (aos, vals)

            def epilogue_chunk(c):
                """Normalize: aos[i] = vals / denom (bf16), denom via
                fast approximate reciprocal."""
                aos, vals = chunk_state[c]
                dnb = work.tile([8, CHUNK], bf16, tag="dnb", name="dnb", bufs=2)
                for h in range(8):
                    nc.gpsimd.dma_start(
                        out=dnb[h : h + 1, :], in_=vals[h][DH : DH + 1, :]
                    )
                dn = work.tile([8, CHUNK], f32, tag="dn", name="dn", bufs=2)
                nc.vector.tensor_copy(dn[:, :], dnb[:, :])
                rc = work.tile([8, CHUNK], f32, tag="rc", name="rc", bufs=2)
                nc.vector.reciprocal_approx_fast(rc[:, :], dn[:, :])
                rcb = work.tile([8, CHUNK], bf16, tag="rcb", name="rcb", bufs=2)
                nc.vector.tensor_copy(rcb[:, :], rc[:, :])
                for h in range(8):
                    rb = work.tile([64, CHUNK], bf16, tag="rb", bufs=4, name="rb")
                    rrow = rcb[h : h + 1, :]
                    rsrc = bass.AP(
                        tensor=rrow.tensor,
                        offset=rrow.offset,
                        ap=[[CHUNK, 1], [0, 64], [1, CHUNK]],
                    )
                    nc.gpsimd.dma_start(out=rb[:, :], in_=rsrc)
                    nc.vector.tensor_mul(
                        aos[h // 2][64 * (h % 2) : 64 * (h % 2) + 64, :],
                        vals[h][0:DH, :],
                        rb[:, :],
                    )
                chunk_state[c] = aos

            # ---- schedule ----
            p0 = proj_fillers(0)
            for u in p0:
                u()

            fillers = [lambda: xc_dma(1)] + proj_fillers(1)
            attention_chunk(0, fillers)
            for u in fillers:
                u()

            for c in range(1, NCHUNK):
                epilogue_chunk(c - 1)
                fl = []
                if c + 1 < NCHUNK:
                    fl.append(lambda c=c: xc_dma(c + 1))
                    fl += proj_fillers(c + 1)
                fl += outproj_fillers(c - 1)
                attention_chunk(c, fl)
                for u in fl:
                    u()
            epilogue_chunk(NCHUNK - 1)
            for u in outproj_fillers(NCHUNK - 1, n_rs=2):
                u()

    nc.finalize()
    return nc


def _get_graph():
    global _GRAPH
    if _GRAPH is None:
        _GRAPH = _build_graph()
    return _GRAPH


def _build_masks():
    # [j, ti] = 1 where ti >= j: token ti attends key j within diagonal block
    return np.ascontiguousarray(np.triu(np.ones((KB, KB), np.float32)))


def _make_in_maps(x, w_qkv, w_out, b_out):
    x = np.asarray(x, np.float32)
    w_qkv = np.asarray(w_qkv, np.float32)
    w_out = np.asarray(w_out, np.float32)
    b_out = np.asarray(b_out, np.float32)
    import ml_dtypes

    bf = ml_dtypes.bfloat16
    xT = [np.ascontiguousarray(x[b].T).astype(bf) for b in range(B)]
    masks = _build_masks().astype(bf)
    hb = np.ascontiguousarray((0.5 * b_out).reshape(1, DM)).astype(np.float32)
    in_maps = []
    for c in range(NCORES):
        b, g = c // 2, c % 2
        in_maps.append(
            {
                "xT": xT[b],
                "wq": np.ascontiguousarray(w_qkv[:, LI * g : LI * (g + 1)]).astype(bf),
                "wk": np.ascontiguousarray(
                    w_qkv[:, DM + LI * g : DM + LI * (g + 1)]
                ).astype(bf),
                "wv": np.ascontiguousarray(
                    w_qkv[:, 2 * DM + LI * g : 2 * DM + LI * (g + 1)]
                ).astype(bf),
                "wo": np.ascontiguousarray(w_out[LI * g : LI * (g + 1), :]).astype(bf),
                "hb": hb,
                "mask": masks,
            }
        )
    return in_maps


def _assemble(results):
    y = np.empty((B, N, DM), np.float32)
    for c in range(NCORES):
        b, g = c // 2, c % 2
        o = results[c]["out"]  # [1024, 1024] of token stripes
        for ch in range(NCHUNK - 1):
            t0 = ch * CHUNK + g * 256
            y[b, t0 : t0 + 256] = o[ch * 256 : (ch + 1) * 256]
        ch = NCHUNK - 1  # last chunk: two half-size ReduceScatter pieces
        for p in range(2):
            t0 = ch * CHUNK + p * 256 + g * 128
            r0 = ch * 256 + p * 128
            y[b, t0 : t0 + 128] = o[r0 : r0 + 128]
    return y


def _install_ntff_hook_shim():
    """The container's antenv package lacks axon_hooks; synthesize it so
    run_bass_kernel_spmd(trace=True) can NTFF-profile via the injected .so."""
    import types

    if "antenv.axon_hooks" in sys.modules:
        return
    try:
        from trn_agent_boot.trn_boot import _ntff_profile_via_ctypes

        hook = _ntff_profile_via_ctypes("/opt/axon/libaxon_pjrt.so")
    except Exception as e:  # profiling degrades, run still works
        print(f"ntff hook shim unavailable: {e}")
        hook = None
    mod = types.ModuleType("antenv.axon_hooks")
    _state = {"hook": hook}
    mod.set_axon_ntff_profile_hook = lambda h: _state.__setitem__("hook", h)
    mod.get_axon_ntff_profile_hook = lambda: _state["hook"]
    sys.modules["antenv.axon_hooks"] = mod
    import antenv

    antenv.axon_hooks = mod


def _run(in_maps, trace=False):
    from concourse import bass_utils

    if trace:
        _install_ntff_hook_shim()
    nc = _get_graph()
    return bass_utils.run_bass_kernel_spmd(
        nc, in_maps, core_ids=list(range(NCORES)), trace=trace
    )


def kernel(x, w_qkv, w_out, b_out):
    res = _run(_make_in_maps(x, w_qkv, w_out, b_out), trace=False)
    return _assemble(res.results)


def kernel_timed(x, w_qkv, w_out, b_out):
    res = _run(_make_in_maps(x, w_qkv, w_out, b_out), trace=True)
    return _assemble(res.results), res


# revision 7
# speedup vs baseline: 1.4551x; 1.0707x over previous
"""Trainium2 distributed causal attention kernel (8 NeuronCores).

Problem: x[4,2048,1024] -> qkv proj -> 16-head causal attention -> out proj.

Sharding (uniform SPMD graph on all 8 cores):
  core c = (batch b = c//2, head-group g = c%2 of 8 heads).
  Each core: projects q/k/v for its 8 heads over the full 2048 tokens of its
  batch (all matmul inputs bf16, host-cast), runs causal flash-style attention
  (no max subtraction -- scores are O(1) for this input distribution),
  computes the partial output projection with its 512 inner dims of w_out,
  adds b_out/2, then a pairwise ReduceScatter(add) over {2b, 2b+1} yields
  final output token-stripes. Host reassembles stripes.

Pipeline: the projection of chunk c+1 and out-projection of chunk c-1 are
interleaved as PE "filler" work inside attention of chunk c, so the PE never
waits on the Scalar engine's EXP chain and stays at full p-state.

Layouts:
  xT   [1024(dm), 2048(tok)] bf16, loaded per 512-token chunk
  kT,qT [512(inner) as 4x[128], 2048] bf16 (2 heads per 128-partition tile)
  v_aug [2048(tok) as 16x[128], 8*65] bf16 (per head: 64 v-cols + ones col)
  sims psum [128(key), 1024(= 2 heads x 512 tok)] = one EXP per head-pair
  pt = exp(sims * 0.125) bf16 [128, 1024], causal masks multiplicative
  pv psum [65, 512] per head accumulates over k-blocks (row 64 = denominator)
  out-proj psum [128(tok), 512(dm col)], bias added during DVE evacuation
"""

import sys

sys.path.insert(0, "/opt/trn_rl_repo")

import numpy as np

B, N, DM = 4, 2048, 1024
H, DH = 16, 64
HG = 8  # heads per core
LI = HG * DH  # local inner = 512
NCORES = 8
CHUNK = 512  # q-chunk tokens
NCHUNK = N // CHUNK  # 4
KB = 128  # k-block size
VW = DH + 1  # v columns per head incl. ones column

_GRAPH = None


def _build_graph():
    from concourse import bacc, bass, mybir, tile

    f32 = mybir.dt.float32
    bf16 = mybir.dt.bfloat16
    Exp = mybir.ActivationFunctionType.Exp

    nc = bacc.Bacc("TRN2", target_bir_lowering=False, debug=False)

    xT_d = nc.dram_tensor("xT", [DM, N], bf16, kind="ExternalInput")
    wq_d = nc.dram_tensor("wq", [DM, LI], bf16, kind="ExternalInput")
    wk_d = nc.dram_tensor("wk", [DM, LI], bf16, kind="ExternalInput")
    wv_d = nc.dram_tensor("wv", [DM, LI], bf16, kind="ExternalInput")
    wo_d = nc.dram_tensor("wo", [LI, DM], bf16, kind="ExternalInput")
    hb_d = nc.dram_tensor("hb", [1, DM], f32, kind="ExternalInput")
    mask_d = nc.dram_tensor("mask", [KB, KB], bf16, kind="ExternalInput")
    out_d = nc.dram_tensor("out", [N // 2, DM], bf16, kind="ExternalOutput")

    RG = [[0, 1], [2, 3], [4, 5], [6, 7]]

    with tile.TileContext(nc) as tc:
        with (
            tc.tile_pool(name="persist", bufs=1) as pers,
            tc.tile_pool(name="work", bufs=4) as work,
            tc.tile_pool(name="aux", bufs=2) as aux,
            tc.tile_pool(name="mmps", bufs=2, space="PSUM") as mmps,
            tc.tile_pool(name="simps", bufs=2, space="PSUM") as simps,
            tc.tile_pool(name="pvps", bufs=2, space="PSUM") as pvps,
            tc.tile_pool(name="dram", bufs=2, space="DRAM") as dram,
        ):
            # ---- persistent constants / weights ----
            # merged tiles: free dim = (d-block, cols); one big DMA per matrix
            wk_all = pers.tile([128, 8 * LI], bf16, tag="wkall", name="wkall")
            wq_all = pers.tile([128, 8 * LI], bf16, tag="wqall", name="wqall")
            wv_all = pers.tile([128, 8 * LI], bf16, tag="wvall", name="wvall")
            xc_all = [
                pers.tile([128, 8 * CHUNK], bf16, tag=f"xc{c}", name=f"xc{c}")
                for c in range(NCHUNK)
            ]

            def wsrc(w_d, cols):
                return bass.AP(
                    tensor=w_d.tensor if hasattr(w_d, "tensor") else w_d,
                    offset=0,
                    ap=[[cols, 128], [128 * cols, 8], [1, cols]],
                )

            def xsrc(c):
                return bass.AP(
                    tensor=xT_d[:, :].tensor,
                    offset=c * CHUNK,
                    ap=[[N, 128], [128 * N, 8], [1, CHUNK]],
                )

            nc.sync.dma_start(out=wk_all[:, :], in_=wsrc(wk_d[:, :], LI))
            nc.sync.dma_start(out=xc_all[0][:, :], in_=xsrc(0))
            nc.sync.dma_start(out=wq_all[:, :], in_=wsrc(wq_d[:, :], LI))
            nc.sync.dma_start(out=wv_all[:, :], in_=wsrc(wv_d[:, :], LI))

            wk_sb = [wk_all[:, d * LI : (d + 1) * LI] for d in range(8)]
            wq_sb = [wq_all[:, d * LI : (d + 1) * LI] for d in range(8)]
            wv_sb = [wv_all[:, d * LI : (d + 1) * LI] for d in range(8)]
            xc = [
                [xc_all[c][:, d * CHUNK : (d + 1) * CHUNK] for d in range(8)]
                for c in range(NCHUNK)
            ]

            mask_sb = pers.tile([KB, KB], bf16, tag="mask", name="mask")
            nc.sync.dma_start(out=mask_sb[:, :], in_=mask_d[:, :])

            wo_all = pers.tile([128, 4 * DM], bf16, tag="woall", name="woall")
            nc.sync.dma_start(
                out=wo_all[:, :],
                in_=bass.AP(
                    tensor=wo_d[:, :].tensor,
                    offset=0,
                    ap=[[DM, 128], [128 * DM, 4], [1, DM]],
                ),
            )
            wo_bf = [wo_all[:, it * DM : (it + 1) * DM] for it in range(4)]

            hb_f = aux.tile([1, DM], f32, tag="hbf", bufs=1, name="hbf")
            nc.sync.dma_start(out=hb_f[:, :], in_=hb_d[:, :])
            # broadcast bias to all 128 partitions once (DMA replication trick)
            bias_bc = pers.tile([128, DM], f32, tag="biasbc", name="biasbc")
            hrow = hb_f[0:1, :]
            hsrc = bass.AP(
                tensor=hrow.tensor,
                offset=hrow.offset,
                ap=[[DM, 1], [0, 128], [1, DM]],
            )
            nc.sync.dma_start(out=bias_bc[:, :], in_=hsrc)

            kT = [pers.tile([128, N], bf16, tag=f"kT{i}", name=f"kT{i}") for i in range(4)]
            qT = [pers.tile([128, N], bf16, tag=f"qT{i}", name=f"qT{i}") for i in range(4)]
            v_aug = [
                pers.tile([128, HG * VW], bf16, tag=f"va{t}", name=f"va{t}")
                for t in range(16)
            ]

            # ---- emission helpers ----
            def xc_dma(c):
                nc.sync.dma_start(out=xc_all[c][:, :], in_=xsrc(c))

            def proj_fillers(c):
                """k/q/v projection of chunk c as a list of PE filler closures
                (each ~4 matmuls; evac on the closing half)."""
                out = []
                for w_sb, dst in ((wk_sb, kT), (wq_sb, qT)):
                    for it in range(4):

                        def f0(w_sb=w_sb, it=it):
                            ps = mmps.tile([128, CHUNK], f32, tag="mm", name="mm")
                            for d in range(4):
                                nc.tensor.matmul(
                                    ps[:, :],
                                    lhsT=w_sb[d][:, it * 128 : (it + 1) * 128],
                                    rhs=xc[c][d][:, :],
                                    start=(d == 0),
                                    stop=False,
                                )
                            return ps

                        def f1(ps_ref, w_sb=w_sb, it=it, dst=dst):
                            ps = ps_ref[0]
                            for d in range(4, 8):
                                nc.tensor.matmul(
                                    ps[:, :],
                                    lhsT=w_sb[d][:, it * 128 : (it + 1) * 128],
                                    rhs=xc[c][d][:, :],
                                    start=False,
                                    stop=(d == 7),
                                )
                            nc.vector.tensor_copy(
                                dst[it][:, c * CHUNK : (c + 1) * CHUNK], ps[:, :]
                            )

                        out.append((f0, f1))
                for tt4 in range(4):
                    tt = 4 * c + tt4

                    def g0(tt4=tt4):
                        ps = mmps.tile([128, CHUNK], f32, tag="mm", name="mm")
                        for d in range(4):
                            nc.tensor.matmul(
                                ps[:, :],
                                lhsT=xc[c][d][:, tt4 * 128 : (tt4 + 1) * 128],
                                rhs=wv_sb[d][:, :],
                                start=(d == 0),
                                stop=False,
                            )
                        return ps

                    def g1(ps_ref, tt=tt, tt4=tt4):
                        ps = ps_ref[0]
                        for d in range(4, 8):
                            nc.tensor.matmul(
                                ps[:, :],
                                lhsT=xc[c][d][:, tt4 * 128 : (tt4 + 1) * 128],
                                rhs=wv_sb[d][:, :],
                                start=False,
                                stop=(d == 7),
                            )
                        va3 = v_aug[tt].rearrange("p (h c) -> p h c", h=HG)
                        nc.vector.memset(va3[:, :, DH : DH + 1], 1.0)
                        nc.vector.tensor_copy(
                            va3[:, :, 0:DH], ps.rearrange("p (h c) -> p h c", h=HG)
                        )

                    out.append((g0, g1))
                # flatten into single-closure units with shared psum handoff
                units = []
                for f0, f1 in out:
                    ps_ref = [None]

                    def u0(f0=f0, ps_ref=ps_ref):
                        ps_ref[0] = f0()

                    def u1(f1=f1, ps_ref=ps_ref):
                        f1(ps_ref)

                    units.append(u0)
                    units.append(u1)
                return units

            chunk_state = {}

            def outproj_fillers(c, n_rs=1):
                """out-projection + RS of chunk c as filler closures."""
                aos = chunk_state[c]
                pd = dram.tile([CHUNK, DM], bf16, tag="pd", name="pd")
                units = []
                ts_per_rs = 4 // n_rs
                for rs_i in range(n_rs):
                    for ts in range(rs_i * ts_per_rs, (rs_i + 1) * ts_per_rs):
                        for ct in range(2):

                            def u(ts=ts, ct=ct):
                                po = mmps.tile([128, CHUNK], f32, tag="mm", name="mm")
                                for it in range(4):
                                    nc.tensor.matmul(
                                        po[:, :],
                                        lhsT=aos[it][:, ts * 128 : (ts + 1) * 128],
                                        rhs=wo_bf[it][:, ct * 512 : (ct + 1) * 512],
                                        start=(it == 0),
                                        stop=(it == 3),
                                    )
                                ob = work.tile(
                                    [128, 512], bf16, tag="ob", name="ob", bufs=2
                                )
                                nc.vector.tensor_add(
                                    ob[:, :], po[:, :],
                                    bias_bc[:, ct * 512 : (ct + 1) * 512],
                                )
                                nc.sync.dma_start(
                                    out=pd[
                                        ts * 128 : (ts + 1) * 128,
                                        ct * 512 : (ct + 1) * 512,
                                    ],
                                    in_=ob[:, :],
                                )

                            units.append(u)

                    def urs(rs_i=rs_i):
                        rows = CHUNK // n_rs
                        rs = dram.tile(
                            [rows // 2, DM], bf16, tag="rs", name="rs",
                            padded_shape=[CHUNK // 2, DM],
                        )
                        nc.gpsimd.collective_compute(
                            "ReduceScatter",
                            mybir.AluOpType.add,
                            replica_groups=RG,
                            ins=[pd[rs_i * rows : (rs_i + 1) * rows, :].opt()],
                            outs=[rs[:, :].opt()],
                        )
                        out_r0 = c * 256 + rs_i * (rows // 2)
                        nc.gpsimd.dma_start(
                            out=out_d[out_r0 : out_r0 + rows // 2, :], in_=rs[:, :]
                        )

                    units.append(urs)
                return units

            def attention_chunk(c, fillers):
                """Causal attention for chunk c; pops filler closures between
                QK and PV emissions to keep the PE dense."""
                nk = 4 * (c + 1)
                nsteps = 4 * nk
                step = [0]

                def drain():
                    step[0] += 1
                    left = nsteps - step[0]
                    k = -(-len(fillers) // (left + 1))  # ceil: spread evenly
                    for _ in range(min(k, len(fillers))):
                        fillers.pop(0)()

                aos = [
                    work.tile([128, CHUNK], bf16, tag=f"ao{i}", name=f"ao{i}", bufs=2)
                    for i in range(4)
                ]
                vals = [None] * 8
                for hp in range(4):
                    pvs = [
                        pvps.tile([VW, CHUNK], f32, tag="pv", name="pv")
                        for _ in range(2)
                    ]
                    sims_of = {}

                    def qk_step(jb):
                        v = jb - (nk - 4)
                        col0 = max(0, v) * KB
                        sims = simps.tile([128, 2 * CHUNK], f32, tag="sim", name="sim")
                        for e in range(2):
                            nc.tensor.matmul(
                                sims[:, e * CHUNK + col0 : (e + 1) * CHUNK],
                                lhsT=kT[hp][
                                    64 * e : 64 * e + 64, jb * KB : (jb + 1) * KB
                                ],
                                rhs=qT[hp][
                                    64 * e : 64 * e + 64,
                                    c * CHUNK + col0 : (c + 1) * CHUNK,
                                ],
                                start=True,
                                stop=True,
                            )
                        sims_of[jb] = (sims, col0)

                    def exp_step(jb):
                        sims, col0 = sims_of[jb]
                        pt = work.tile(
                            [128, 2 * CHUNK], bf16, tag="pt", name="pt", bufs=3
                        )
                        if col0 > 0:
                            pt3 = pt.rearrange("p (e t) -> p e t", e=2)
                            nc.vector.memset(pt3[:, :, 0:col0], 0.0)
                            s3 = sims.rearrange("p (e t) -> p e t", e=2)
                            nc.scalar.activation(
                                pt3[:, :, col0:CHUNK],
                                s3[:, :, col0:CHUNK],
                                Exp,
                                scale=float(DH**-0.5),
                            )
                        else:
                            nc.scalar.activation(
                                pt[:, :], sims[:, :], Exp, scale=float(DH**-0.5)
                            )
                        v = jb - (nk - 4)
                        if v >= 0:
                            for e in range(2):
                                nc.vector.tensor_mul(
                                    pt[:, e * CHUNK + col0 : e * CHUNK + col0 + KB],
                                    pt[:, e * CHUNK + col0 : e * CHUNK + col0 + KB],
                                    mask_sb[:, :],
                                )
                        sims_of[jb] = pt

                    def pv_step(jb):
                        pt = sims_of.pop(jb)
                        for e in range(2):
                            h = 2 * hp + e
                            nc.tensor.matmul(
                                pvs[e][:, :],
                                lhsT=v_aug[jb][:, h * VW : (h + 1) * VW],
                                rhs=pt[:, e * CHUNK : (e + 1) * CHUNK],
                                start=(jb == 0),
                                stop=(jb == nk - 1),
                            )

                    qk_step(0)
                    exp_step(0)
                    for jb in range(1, nk):
                        qk_step(jb)
                        exp_step(jb)
                        drain()
                        pv_step(jb - 1)
                    drain()
                    pv_step(nk - 1)

                    # evacuate PV psum (values + denominator row) to SBUF
                    for e in range(2):
                        h = 2 * hp + e
                        t = work.tile(
                            [VW, CHUNK], bf16, tag=f"pvsb{h}", bufs=2, name=f"pvsb{h}"
                        )
                        nc.vector.tensor_copy(t[:, :], pvs[e][:, :])
                        vals[h] = t

                chunk_state[c] = (aos, vals)

            def epilogue_chunk(c):
                """Normalize: aos[i] = vals / denom (bf16), denom via
                fast approximate reciprocal."""
                aos, vals = chunk_state[c]
                dnb = work.tile([8, CHUNK], bf16, tag="dnb", name="dnb", bufs=2)
                for h in range(8):
                    nc.sync.dma_start(
                        out=dnb[h : h + 1, :], in_=vals[h][DH : DH + 1, :]
                    )
                dn = work.tile([8, CHUNK], f32, tag="dn", name="dn", bufs=2)
                nc.vector.tensor_copy(dn[:, :], dnb[:, :])
                rc = work.tile([8, CHUNK], f32, tag="rc", name="rc", bufs=2)
                nc.vector.reciprocal_approx_fast(rc[:, :], dn[:, :])
                rcb = work.tile([8, CHUNK], bf16, tag="rcb", name="rcb", bufs=2)
                nc.vector.tensor_copy(rcb[:, :], rc[:, :])
                for h in range(8):
                    rb = work.tile([64, CHUNK], bf16, tag="rb", bufs=4, name="rb")
                    rrow = rcb[h : h + 1, :]
                    rsrc = bass.AP(
                        tensor=rrow.tensor,
                        offset=rrow.offset,
                        ap=[[CHUNK, 1], [0, 64], [1, CHUNK]],
                    )
                    nc.sync.dma_start(out=rb[:, :], in_=rsrc)
                    nc.vector.tensor_mul(
                        aos[h // 2][64 * (h % 2) : 64 * (h % 2) + 64, :],
                        vals[h][0:DH, :],
                        rb[:, :],
                    )
                chunk_state[c] = aos

            # ---- schedule ----
            p0 = proj_fillers(0)
            for u in p0:
                u()

            xc_dma(1)
            fillers = proj_fillers(1)
            attention_chunk(0, fillers)
            for u in fillers:
                u()

            for c in range(1, NCHUNK):
                if c + 1 < NCHUNK:
                    xc_dma(c + 1)
                epilogue_chunk(c - 1)
                fl = []
                if c + 1 < NCHUNK:
                    fl += proj_fillers(c + 1)
                fl += outproj_fillers(c - 1)
                attention_chunk(c, fl)
                for u in fl:
                    u()
            epilogue_chunk(NCHUNK - 1)
            for u in outproj_fillers(NCHUNK - 1, n_rs=1):
                u()

    nc.finalize()
    return nc


def _get_graph():
    global _GRAPH
    if _GRAPH is None:
        _GRAPH = _build_graph()
    return _GRAPH


def _build_masks():
    # [j, ti] = 1 where ti >= j: token ti attends key j within diagonal block
    return np.ascontiguousarray(np.triu(np.ones((KB, KB), np.float32)))


def _make_in_maps(x, w_qkv, w_out, b_out):
    x = np.asarray(x, np.float32)
    w_qkv = np.asarray(w_qkv, np.float32)
    w_out = np.asarray(w_out, np.float32)
    b_out = np.asarray(b_out, np.float32)
    import ml_dtypes

    bf = ml_dtypes.bfloat16
    xT = [np.ascontiguousarray(x[b].T).astype(bf) for b in range(B)]
    masks = _build_masks().astype(bf)
    hb = np.ascontiguousarray((0.5 * b_out).reshape(1, DM)).astype(np.float32)
    in_maps = []
    for c in range(NCORES):
        b, g = c // 2, c % 2
        in_maps.append(
            {
                "xT": xT[b],
                "wq": np.ascontiguousarray(w_qkv[:, LI * g : LI * (g + 1)]).astype(bf),
                "wk": np.ascontiguousarray(
                    w_qkv[:, DM + LI * g : DM + LI * (g + 1)]
                ).astype(bf),
                "wv": np.ascontiguousarray(
                    w_qkv[:, 2 * DM + LI * g : 2 * DM + LI * (g + 1)]
                ).astype(bf),
                "wo": np.ascontiguousarray(w_out[LI * g : LI * (g + 1), :]).astype(bf),
                "hb": hb,
                "mask": masks,
            }
        )
    return in_maps


def _assemble(results):
    y = np.empty((B, N, DM), np.float32)
    for c in range(NCORES):
        b, g = c // 2, c % 2
        o = np.asarray(results[c]["out"], np.float32)  # [1024,1024] token stripes
        for ch in range(NCHUNK - 1):
            t0 = ch * CHUNK + g * 256
            y[b, t0 : t0 + 256] = o[ch * 256 : (ch + 1) * 256]
        ch = NCHUNK - 1
        t0 = ch * CHUNK + g * 256
        y[b, t0 : t0 + 256] = o[ch * 256 : (ch + 1) * 256]
    return y


def _install_ntff_hook_shim():
    """The container's antenv package lacks axon_hooks; synthesize it so
    run_bass_kernel_spmd(trace=True) can NTFF-profile via the injected .so."""
    import types

    if "antenv.axon_hooks" in sys.modules:
        return
    try:
        from trn_agent_boot.trn_boot import _ntff_profile_via_ctypes

        hook = _ntff_profile_via_ctypes("/opt/axon/libaxon_pjrt.so")
    except Exception as e:  # profiling degrades, run still works
        print(f"ntff hook shim unavailable: {e}")
        hook = None
    mod = types.ModuleType("antenv.axon_hooks")
    _state = {"hook": hook}
    mod.set_axon_ntff_profile_hook = lambda h: _state.__setitem__("hook", h)
    mod.get_axon_ntff_profile_hook = lambda: _state["hook"]
    sys.modules["antenv.axon_hooks"] = mod
    import antenv

    antenv.axon_hooks = mod


def _run(in_maps, trace=False):
    from concourse import bass_utils

    if trace:
        _install_ntff_hook_shim()
    nc = _get_graph()
    return bass_utils.run_bass_kernel_spmd(
        nc, in_maps, core_ids=list(range(NCORES)), trace=trace
    )


def kernel(x, w_qkv, w_out, b_out):
    res = _run(_make_in_maps(x, w_qkv, w_out, b_out), trace=False)
    return _assemble(res.results)


def kernel_timed(x, w_qkv, w_out, b_out):
    res = _run(_make_in_maps(x, w_qkv, w_out, b_out), trace=True)
    return _assemble(res.results), res
